# revision 15
# baseline (speedup 1.0000x reference)
"""Bass/Trainium2 kernel for bidirectional Chamfer loss.

Problem: y_true [8, 8192, 3], y_pred [8, 8192, 3] fp32 ->
  scalar = mean_b(sum_n min_m d2[b,n,m]) + mean_b(sum_m min_n d2[b,n,m])
  with d2 = max(|x|^2 + |y|^2 - 2 x.y, 0).

Strategy (windowed fused-kNN, exact):
  - Data-parallel over batch: 8 batches -> 8 NeuronCores.
  - Host sorts both clouds by z. Since both clouds are drawn from the same
    distribution, quantile-aligned index windows capture nearly every
    nearest neighbor: row block i (128 sorted x points) only scans the
    W=512 sorted-y columns centered at its quantile, cutting the drained
    [8192, 8192] distance matrix ~12x.
  - Exactness: a cheap host pass (index window Wh=256) upper-bounds each
    point's NN distance u'. If the [z +- sqrt(u')] interval lies inside the
    point's device window, the windowed min is provably exact. The few
    points that fail this test (<=~90 per batch/direction) are gathered as
    128 duplicated "hard" x rows scanning ALL columns, and 128 duplicated
    "hard" y columns seen by ALL row blocks, so their mins are exact too.
    A final host safety net re-verifies every point with the device minima
    and brute-forces any stragglers (structurally none) in numpy.
  - The matmul computes NEGATED distances -d2 via a K=24 bf16 triple-split
    contraction (fp32-accurate; PE runs at its 1-cycle/column bf16 rate),
    so all device reductions are max (required: partition_all_reduce
    supports only max/add).
  - Per row block: PE writes -d2 into PSUM; ScalarE converts PSUM -> fp16
    SBUF; DVE folds the window to its row max (fwd) and keeps a running
    elementwise max R[128, 8320] across blocks via in-place TT max (bwd).
    GPSIMD retires finalized 1024-column stripes of R during the sweep
    with partition_all_reduce (cross-partition max) -> bwd minima, no
    DMA-transpose endgame.
  - Host epilogue: negate, relu, overwrite suspect entries with their
    exact hard-row/extra-column results, fp64 sums, mean over batch.
"""

import numpy as np
import ml_dtypes

N = 8192  # points per cloud
D = 3
K = 24  # contraction lanes of the augmented matmul
PART = 128  # rows of the distance matrix per tile
NB = N // PART  # 64 row blocks
W = 512  # sorted-column window per row block
E = 128  # hard-y extra columns (suspects, padded)
HARD = 128  # hard-x rows (suspects, padded) = one extra row block
NX = N + HARD  # stationary points incl. hard rows
NY = N + E  # moving points incl. extra columns
CW = W + E  # windowed-block column count
HCW = 1024  # hard-block chunk width
STRIPE = 1024  # columns retired per partition_all_reduce

# window starts per row block (compile-time constants, shared host/device)
CS = np.clip(np.arange(NB) * PART + PART // 2 - W // 2, 0, N - W)
# hard-block chunks (start, width) covering all NY columns
CHUNKS = [(c, min(HCW, NY - c)) for c in range(0, NY, HCW)]
# first block index whose window no longer overlaps stripe s
STRIPE_AFTER = {
    s: int(np.searchsorted(CS, STRIPE * (s + 1)))
    for s in range(N // STRIPE - 1)  # stripes 0..6 retire mid-sweep
}

_BF16 = ml_dtypes.bfloat16
MARGIN = 1.05
WH = 256  # host bound pass window


def _split3(a):
    """fp32 -> three bf16 limbs with a ~= hi+mid+lo to ~2^-24 relative."""
    a = np.ascontiguousarray(a, np.float32)
    hi = a.astype(_BF16)
    r1 = a - hi.astype(np.float32)
    mid = r1.astype(_BF16)
    r2 = r1 - mid.astype(np.float32)
    lo = r2.astype(_BF16)
    return hi, mid, lo


def _build_sides(stat, mov, stat_sq, mov_sq):
    """Build [K, n] bf16 stationary (lhsT) / moving (rhs) lane matrices.

    lane i contributes A[i, n] * B[i, m] to PSUM[n, m]; the 24 lanes sum to
    stat_sq[n] + mov_sq[m] - 2 * stat[n].mov[m] at ~fp32 accuracy.
    """
    A = np.zeros((K, stat.shape[0]), _BF16)
    B = np.zeros((K, mov.shape[0]), _BF16)
    t = (-2.0 * mov.astype(np.float64)).astype(np.float32)
    for d in range(D):
        xh, xm, xl = _split3(stat[:, d])
        th, tm, tl = _split3(t[:, d])
        r = 6 * d
        A[r + 0], B[r + 0] = xh, th
        A[r + 1], B[r + 1] = xh, tm
        A[r + 2], B[r + 2] = xm, th
        A[r + 3], B[r + 3] = xm, tm
        A[r + 4], B[r + 4] = xh, tl
        A[r + 5], B[r + 5] = xl, th
    sh, sm, sl = _split3(mov_sq)
    A[18:21] = _BF16(1.0)
    B[18], B[19], B[20] = sh, sm, sl
    qh, qm, ql = _split3(stat_sq)
    A[21], A[22], A[23] = qh, qm, ql
    B[21:24] = _BF16(1.0)
    return A, B


def _host_bound(xs, ys, wh=WH):
    """u'_p >= true NN d2 of xs[p] vs ys: min over the index-aligned
    [p-wh/2, p+wh/2) window of sorted ys (both clouds z-sorted)."""
    n = xs.shape[0]
    u = np.empty(n, np.float32)
    step = 512
    for p0 in range(0, n, step):
        p1 = min(n, p0 + step)
        lo = max(0, p0 - wh // 2)
        hi = min(n, p1 + wh // 2)
        xx = xs[p0:p1]
        yy = ys[lo:hi]
        d2 = (
            (xx**2).sum(1)[:, None]
            + (yy**2).sum(1)[None, :]
            - 2.0 * xx @ yy.T
        )
        idx = np.arange(lo, hi)[None, :] - np.arange(p0, p1)[:, None]
        np.copyto(d2, np.inf, where=(idx < -wh // 2) | (idx >= wh // 2))
        u[p0:p1] = np.maximum(d2.min(1), 0.0)
    return u


def _fwd_suspects(xs, ys, u):
    """x points whose W-window (per CS) provably-contains-NN test fails."""
    r = np.sqrt(u) * MARGIN + 1e-3
    zx = xs[:, 2]
    zy = ys[:, 2]
    blk = np.arange(N) // PART
    c = CS[blk]
    miss_lo = (zx - r < zy[c]) & (c > 0)
    miss_hi = (zx + r > zy[c + W - 1]) & (c < N - W)
    return miss_lo | miss_hi


def _bwd_suspects(xs, ys, u):
    """y points whose covering x-row range test fails. Coverage of sorted-y
    position q = x rows of blocks {i : CS[i] <= q < CS[i]+W}."""
    r = np.sqrt(u) * MARGIN + 1e-3
    zy = ys[:, 2]
    zx = xs[:, 2]
    q = np.arange(N)
    i_min = np.searchsorted(CS, q - W, side="right")  # first i: CS[i] > q-W
    i_max = np.searchsorted(CS, q, side="right") - 1  # last i: CS[i] <= q
    row_lo = i_min * PART
    row_hi = np.minimum(i_max * PART + PART, N)
    miss_lo = (zy - r < zx[row_lo]) & (row_lo > 0)
    miss_hi = (zy + r > zx[row_hi - 1]) & (row_hi < N)
    return miss_lo | miss_hi


def _pad_idx(idx, size):
    idx = np.asarray(idx, np.int64)[:size]
    if idx.size < size:
        idx = np.concatenate([idx, np.zeros(size - idx.size, np.int64)])
    return idx


_NC_CACHE = {}


def _build_fast(repeat=1):
    """Windowed sweep program: hard block (7 full-width chunks) + 64
    windowed blocks + striped partition_all_reduce bwd retirement."""
    key = ("fast", repeat)
    if key in _NC_CACHE:
        return _NC_CACHE[key]

    from concourse import bacc, mybir, bass_isa
    import concourse.tile as tile

    nc = bacc.Bacc("TRN2", target_bir_lowering=False, debug=False)
    f32 = mybir.dt.float32
    f16 = mybir.dt.float16
    bf16 = mybir.dt.bfloat16

    a_in = nc.dram_tensor("af", [K, NX], bf16, kind="ExternalInput")
    b_in = nc.dram_tensor("bf", [K, NY], bf16, kind="ExternalInput")
    fwd_out = nc.dram_tensor("fwdmin", [PART, NB + 1], f32, kind="ExternalOutput")
    bwd_out = nc.dram_tensor("bwdmin", [1, NY], f32, kind="ExternalOutput")

    def tt_max(out_ap, a_ap, b_ap):
        eng = nc.vector
        return eng.add_instruction(
            mybir.InstTensorTensor(
                name=nc.get_next_instruction_name(),
                op=mybir.AluOpType.max,
                ins=[eng.lower_ap(a_ap), eng.lower_ap(b_ap)],
                outs=[eng.lower_ap(out_ap)],
            )
        )

    def tr_max(out_ap, in_ap):
        nc.vector.tensor_reduce(
            out=out_ap, in_=in_ap, axis=mybir.AxisListType.X,
            op=mybir.AluOpType.max,
        )

    def mm_splits(w):
        splits = []
        c = 0
        while c < w:
            s = min(512, w - c)
            splits.append((c, s))
            c += s
        return splits

    with tile.TileContext(nc) as tc:
        with (
            tc.tile_pool(name="lanes", bufs=1) as lanes,
            tc.tile_pool(name="rpool", bufs=1) as rpool,
            tc.tile_pool(name="psum", bufs=int(os.environ.get("CHAMFER_PSUM_BUFS", "3")), space="PSUM") as psum,
            tc.tile_pool(name="conv", bufs=int(os.environ.get("CHAMFER_CONV_BUFS", "3")) ) as conv_pool,
            tc.tile_pool(name="fold", bufs=2) as fold_pool,
            tc.tile_pool(name="mins", bufs=1) as mins_pool,
            tc.tile_pool(name="parp", bufs=2) as par_pool,
        ):
            a_sb = lanes.tile([K, NX], bf16, tag="af")
            nc.sync.dma_start(out=a_sb[:], in_=a_in[:])
            b_sb = lanes.tile([K, NY], bf16, tag="bf")
            nc.sync.dma_start(out=b_sb[:], in_=b_in[:])
            R = rpool.tile([PART, NY], f16, tag="R")
            fwdmins = mins_pool.tile([PART, NB + 1], f32, tag="fwdmin")
            hardcm = mins_pool.tile([PART, len(CHUNKS)], f32, tag="hardcm")

            for rep in range(repeat):
                # --- hard block: rows N..N+128 scan every column; the ACT
                # convert doubles as R's first touch ---
                lhsT = a_sb[:, N : N + PART]
                for ci, (c0, cw) in enumerate(CHUNKS if "hard" not in ABL else CHUNKS[:1]):
                    ps = psum.tile([PART, HCW], f32)
                    for o, s in mm_splits(cw):
                        nc.tensor.matmul(
                            ps[:, o : o + s], lhsT, b_sb[:, c0 + o : c0 + o + s],
                            start=True, stop=True,
                        )
                    nc.scalar.copy(out=R[:, c0 : c0 + cw], in_=ps[:, 0:cw])
                    if cw >= 1024:
                        th = fold_pool.tile([PART, cw // 2], f16, tag="th")
                        tt_max(
                            th[:], R[:, c0 : c0 + cw // 2],
                            R[:, c0 + cw // 2 : c0 + cw],
                        )
                        tr_max(hardcm[:, ci : ci + 1], th[:])
                    else:
                        tr_max(hardcm[:, ci : ci + 1], R[:, c0 : c0 + cw])
                tr_max(fwdmins[:, NB : NB + 1], hardcm[:])

                # --- windowed sweep, QB row blocks per iteration ---
                import os
                QB = int(os.environ.get("CHAMFER_QB", "4"))
                ABL = os.environ.get("CHAMFER_ABLATE", "")
                for i in range(0, NB, QB):
                    bufp = conv_pool.tile([PART, QB * CW], f16, tag="buf")
                    for r_i in range(QB):
                        c = int(CS[i + r_i])
                        lhsT = a_sb[:, (i + r_i) * PART : (i + r_i + 1) * PART]
                        ps = psum.tile([PART, HCW], f32)
                        for o, s in mm_splits(W):
                            nc.tensor.matmul(
                                ps[:, o : o + s], lhsT,
                                b_sb[:, c + o : c + o + s],
                                start=True, stop=True,
                            )
                        nc.tensor.matmul(
                            ps[:, W : W + E], lhsT, b_sb[:, N : N + E],
                            start=True, stop=True,
                        )
                        nc.scalar.copy(
                            out=bufp[:, r_i * CW : (r_i + 1) * CW],
                            in_=ps[:, 0:CW],
                        )
                    # fwd fold for all rows via strided views [p, QB, w]
                    b3 = bufp[:].rearrange("p (rb w) -> p rb w", rb=QB)
                    if "fold" in ABL:
                        t23 = b3[:, :, 0 : W // 4]
                    else:
                        t1 = fold_pool.tile([PART, QB * W // 2], f16, tag="t1")
                        t13 = t1[:].rearrange("p (rb w) -> p rb w", rb=QB)
                        tt_max(t13, b3[:, :, 0 : W // 2], b3[:, :, W // 2 : W])
                        t2 = fold_pool.tile([PART, QB * W // 4], f16, tag="t2")
                        t23 = t2[:].rearrange("p (rb w) -> p rb w", rb=QB)
                        tt_max(t23, t13[:, :, 0 : W // 4], t13[:, :, W // 4 :])
                    if "fwd" not in ABL:
                        tr_max(fwdmins[:, i : i + QB], t23)
                    # bwd window updates: in-place running max into R
                    if "rwin" not in ABL:
                        for r_i in range(QB):
                            c = int(CS[i + r_i])
                            tt_max(
                                R[:, c : c + W], R[:, c : c + W],
                                bufp[:, r_i * CW : r_i * CW + W],
                            )
                    # bwd extras: combine the QB extras slices by a
                    # strided-view halving tree, then one in-place R update
                    ep = fold_pool.tile([PART, (QB // 2) * E], f16, tag="ep")
                    ep3 = ep[:].rearrange("p (rb w) -> p rb w", rb=QB // 2)
                    tt_max(
                        ep3,
                        b3[:, 0 : QB : 2, W:CW],
                        b3[:, 1 : QB : 2, W:CW],
                    )
                    nb_e = QB // 2
                    while nb_e > 1:
                        en = fold_pool.tile(
                            [PART, (nb_e // 2) * E], f16, tag=f"e{nb_e}"
                        )
                        en3 = en[:].rearrange(
                            "p (rb w) -> p rb w", rb=nb_e // 2
                        )
                        ec = ep[:].rearrange("p (rb w) -> p rb w", rb=nb_e)
                        tt_max(en3, ec[:, 0 : nb_e : 2, :], ec[:, 1 : nb_e : 2, :])
                        ep = en
                        nb_e //= 2
                    tt_max(R[:, N : N + E], R[:, N : N + E], ep[:])
                    # retire finalized stripes via cross-partition max
                    for s, after in STRIPE_AFTER.items():
                        if i < after <= i + QB:
                            par = par_pool.tile([PART, STRIPE], f32, tag="par")
                            nc.gpsimd.partition_all_reduce(
                                par[:], R[:, s * STRIPE : (s + 1) * STRIPE],
                                channels=PART, reduce_op=bass_isa.ReduceOp.max,
                            )
                            nc.sync.dma_start(
                                out=bwd_out[0:1, s * STRIPE : (s + 1) * STRIPE],
                                in_=par[0:1, :],
                            )
                # tail: last stripe + extras
                s_tail = N // STRIPE - 1
                par = par_pool.tile([PART, STRIPE], f32, tag="par")
                nc.gpsimd.partition_all_reduce(
                    par[:], R[:, s_tail * STRIPE : N],
                    channels=PART, reduce_op=bass_isa.ReduceOp.max,
                )
                nc.sync.dma_start(
                    out=bwd_out[0:1, s_tail * STRIPE : N], in_=par[0:1, :]
                )
                par_e = par_pool.tile([PART, E], f32, tag="par_e")
                nc.gpsimd.partition_all_reduce(
                    par_e[:], R[:, N:NY],
                    channels=PART, reduce_op=bass_isa.ReduceOp.max,
                )
                nc.sync.dma_start(out=bwd_out[0:1, N:NY], in_=par_e[0:1, :])
            nc.sync.dma_start(out=fwd_out[:], in_=fwdmins[:])

    nc.compile()
    _NC_CACHE[key] = nc
    return nc


def _prep_batch(xb, yb):
    """Sort, find suspects, build augmented negated lane matrices."""
    xord = np.argsort(xb[:, 2], kind="stable")
    yord = np.argsort(yb[:, 2], kind="stable")
    xs = xb[xord]
    ys = yb[yord]
    ux = _host_bound(xs, ys)
    uy = _host_bound(ys, xs)
    fs = np.flatnonzero(_fwd_suspects(xs, ys, ux))
    bs = np.flatnonzero(_bwd_suspects(xs, ys, uy))
    hardx = _pad_idx(fs, HARD)
    hardy = _pad_idx(bs, E)
    X_aug = np.concatenate([xs, xs[hardx]], 0)
    Y_aug = np.concatenate([ys, ys[hardy]], 0)
    x2 = (X_aug.astype(np.float64) ** 2).sum(1).astype(np.float32)
    y2 = (Y_aug.astype(np.float64) ** 2).sum(1).astype(np.float32)
    A, B = _build_sides(X_aug, Y_aug, x2, y2)
    B = -B  # PSUM accumulates -d2 so device reductions are max
    return {
        "in_map": {"af": A, "bf": B},
        "xs": xs,
        "ys": ys,
        "hardx": hardx,
        "hardy": hardy,
        "n_fs": fs.size,
        "n_bs": bs.size,
    }


def _epilogue(prep, fwdmin, bwdmin):
    """Merge device results to exact per-point minima; verify; sum."""
    xs, ys = prep["xs"], prep["ys"]
    # fwd: [128, 64] block-major -> sorted order; negate back; relu
    fwd = np.maximum(-fwdmin[:, 0:NB].T.reshape(N).astype(np.float64), 0.0)
    hard_vals = np.maximum(-fwdmin[:, NB].astype(np.float64), 0.0)
    fwd[prep["hardx"]] = hard_vals  # exact full-scan results
    bwd = np.maximum(-bwdmin[0, 0:N].astype(np.float64), 0.0)
    extra_vals = np.maximum(-bwdmin[0, N:NY].astype(np.float64), 0.0)
    np.minimum.at(bwd, prep["hardy"], extra_vals)  # exact results

    # safety net: re-verify the window-containment test with the actual
    # device minima; brute-force any point that still fails (expected: none)
    hardx_set = np.zeros(N, bool)
    hardx_set[prep["hardx"]] = True
    viol = np.flatnonzero(
        _fwd_suspects(xs, ys, fwd.astype(np.float32)) & ~hardx_set
    )
    for p in viol:
        d2 = ((xs[p] - ys) ** 2).sum(1)
        fwd[p] = max(float(d2.min()), 0.0)
    hardy_set = np.zeros(N, bool)
    hardy_set[prep["hardy"]] = True
    violb = np.flatnonzero(
        _bwd_suspects(xs, ys, bwd.astype(np.float32)) & ~hardy_set
    )
    for q in violb:
        d2 = ((ys[q] - xs) ** 2).sum(1)
        bwd[q] = max(float(d2.min()), 0.0)
    return fwd.sum() + bwd.sum(), viol.size + violb.size


def kernel(y_true: np.ndarray, y_pred: np.ndarray) -> np.ndarray:
    from concourse import bass_utils

    x = np.asarray(y_true, np.float32)
    y = np.asarray(y_pred, np.float32)
    B = x.shape[0]

    preps = [_prep_batch(x[b], y[b]) for b in range(B)]
    nc = _build_fast()
    results = bass_utils.run_bass_kernel_spmd(
        nc, [p["in_map"] for p in preps], core_ids=list(range(B))
    ).results

    total = 0.0
    for b in range(B):
        s, nviol = _epilogue(preps[b], results[b]["fwdmin"], results[b]["bwdmin"])
        total += s
    return np.asarray(total / B, dtype=np.float32)
